# revision 28
# baseline (speedup 1.0000x reference)
"""Trainium2 Bass kernel for nn_Attention_9998683865539.

Multi-head attention (B=8, N=1024, C=768, H=12, HD=64, fp32), data-parallel
over the batch across 8 NeuronCores (one batch element per core, weights
replicated, no collectives). All matmul operands are bf16 (inputs are
pre-cast on the host, halving HBM traffic); PSUM accumulation is fp32, so
end-to-end error vs the fp32 reference is ~6e-3 relative (gate: 2e-2).

Per-core dataflow:
  qkT  = (w_qkv_scaled.T).T @ xT    feature-major; q-rows of w_qkv are
                                    pre-scaled by HD^-0.5 on the host.
                                    q blocks stay packed (2 heads per 128
                                    partitions); k blocks are unpacked into
                                    per-head [128, N] bf16 tiles whose
                                    complementary 64 partitions are EXACTLY
                                    ZERO (one-time memset outside the loop).
  v    = x @ w_v.T                  token-major, packed per head with a
                                    trailing ones column (V' = [v | 1])
  per head h:
    S.T[k, q] = k_pad_h.T @ q_packed    K=128 full-array matmul: the zero
                                    rows of k select just head h while the
                                    partner head's q rows contribute 0.
                                    (A K=64 matmul runs at HALF rate on the
                                    128x128 PE unless row-tiled, and mode
                                    switches between 64/128-row tiling cost
                                    ~0.5us, so zero-padding to K=128 beats
                                    both alternatives in this schedule.)
    P.T = exp(S.T)                  ScalarE, no max-subtraction (|S|<~7);
                                    attn@v lags exps by one k-tile so its
                                    pt input is ready when the PE arrives
    [U.T; den] = V'.T @ P.T         M=65 matmul, PSUM accum over k-tiles;
                                    row 64 is the softmax denominator
    attnT_h = U.T * bcast(1/den)    DVE reciprocal + GPSIMD
                                    partition_broadcast + DVE multiply
  out = attnT.T @ w_proj.T + b_proj    bias added on DVE, out streamed f32

qk feature blocks 1..5, 7..11 are emitted as "filler" matmul chunks spliced
between head k-tile iterations so the PE stays busy while ACT drains exps.
One-time work (ACT exp-table warm, PE spin-up, constants, qk_k zeroing,
bias load) is hoisted out of the For_i repeat loop.
"""
import sys

sys.path.insert(0, "/opt/trn_rl_repo")

import collections

import numpy as np

import concourse.bass as bass
import concourse.tile as tile
from concourse import bacc, mybir
from concourse import bass_utils

F32 = mybir.dt.float32
F32R = mybir.dt.float32r
BF16 = mybir.dt.bfloat16
EXP = mybir.ActivationFunctionType.Exp
MULT = mybir.AluOpType.mult

B = 8            # batch (one element per core)
C = 768          # channels
N = 1024         # tokens
H = 12           # heads
HD = 64          # head dim
SCALE = HD ** -0.5
NCT = C // 128   # 6 channel tiles
NTT = N // 128   # 8 token tiles
NQK = 12         # qk feature tiles (1536/128)
WV = H * (HD + 1)  # 780: per token-tile, 12 heads x (64 v + 1 ones)


def _build(reps=0, pt_bufs=6, wqs_bufs=6):
    nc = bacc.Bacc("TRN2", target_bir_lowering=False, debug=False)

    xT_d = nc.dram_tensor("xT", [C, N], BF16, kind="ExternalInput").ap()
    wqb_d = nc.dram_tensor("wqb", [NQK, 128, C], BF16, kind="ExternalInput").ap()
    wv_d = nc.dram_tensor("wv", [C, C], BF16, kind="ExternalInput").ap()
    wp_d = nc.dram_tensor("wp", [C, C], BF16, kind="ExternalInput").ap()
    bp_d = nc.dram_tensor("bp", [128, C], F32, kind="ExternalInput").ap()
    out_d = nc.dram_tensor("out", [N, C], F32, kind="ExternalOutput").ap()

    with tile.TileContext(nc) as tc:
        with (
            tc.tile_pool(name="big", bufs=1) as big,
            tc.tile_pool(name="ptp", bufs=pt_bufs) as ptp,
            tc.tile_pool(name="wkp", bufs=1) as wkp,
            tc.tile_pool(name="psp", bufs=2, space=bass.MemorySpace.PSUM) as psp,
        ):
            qk_q = big.tile([128, NCT * N], BF16)     # 12KB/part: q blocks, packed
            qk_k = big.tile([128, H * N], BF16)       # 24KB/part: per-head k,
            #   complementary 64 partitions exactly zero so K=128 matmuls
            #   against packed q blocks select just this head's features
            vp_t = big.tile([128, NTT * WV], BF16)    # 12.2KB/part
            attnT = big.tile([128, NCT * N], BF16)    # 12KB/part
            xr = big.tile([128, NCT * N], BF16)       # 12KB/part
            wv_t = big.tile([128, NCT * C], BF16)     # 9KB/part
            wp_t = big.tile([128, NCT * C], BF16)     # 9KB/part
            ones12 = wkp.tile([128, H], F32)
            bias_sb = wkp.tile([128, C], F32)
            warm = wkp.tile([128, 1], F32)

            def warmup():
                # one-time: constants, ACT exp-table load, PE spin-up, bias
                nc.vector.memset(ones12[:], 1.0)
                nc.scalar.activation(warm[:], ones12[:, 0:1], EXP)
                ps_w = psp.tile([128, N], F32, tag="s", name="ps_warm")
                for _ in range(30):
                    nc.tensor.matmul(
                        ps_w[0:H, 0:H], ones12[:], ones12[:],
                        start=True, stop=True,
                    )
                nc.sync.dma_start(bias_sb[:], bp_d[:])
                # zero qk_k so each head's complementary 64 partitions stay
                # exactly zero (in-loop evacuations only write the data half)
                nc.vector.memset(qk_k[:], 0.0)

            def emit():
                def wq_load(ft):
                    wqs = wkp.tile(
                        [128, NCT * 128], BF16, tag="wqs", bufs=wqs_bufs
                    )
                    nc.gpsimd.dma_start(wqs[:], wqb_d[ft])
                    return wqs

                # DMA order: wq block 0 (per-ct chunks interleaved with xT
                # chunks) first, then block 6, then wv; wp + rest trail.
                w_first = wkp.tile([128, NCT * 128], BF16, tag="wqs", bufs=wqs_bufs, name="w_first")
                nc.gpsimd.dma_start(w_first[:], wqb_d[0])
                for ct in range(NCT):
                    nc.sync.dma_start(
                        xr[:, N * ct : N * (ct + 1)],
                        xT_d[128 * ct : 128 * (ct + 1), :],
                    )
                w_second = wq_load(6)
                nc.sync.dma_start(
                    wv_t[:].rearrange("p (ct f) -> p ct f", f=C),
                    wv_d[:].rearrange("(ct p) f -> p ct f", p=128),
                )

                def qk_evac(ft, ps):
                    if ft < NCT:  # q block: packed bf16
                        nc.vector.tensor_copy(
                            qk_q[:, N * ft : N * (ft + 1)], ps[:]
                        )
                    else:  # k block: unpack into per-head zero-padded blocks
                        t = ft - NCT
                        nc.vector.tensor_copy(
                            qk_k[0:64, N * 2 * t : N * (2 * t + 1)], ps[0:64, :]
                        )
                        nc.vector.tensor_copy(
                            qk_k[64:128, N * (2 * t + 1) : N * (2 * t + 2)],
                            ps[64:128, :],
                        )

                def qk_compute(ft, wqs):
                    """qkT block ft, monolithic (pre-head phase)."""
                    ps = psp.tile([128, N], F32, tag="s")
                    for ct in range(NCT):
                        lhs = wqs[:, 128 * ct : 128 * (ct + 1)]
                        for qh in range(2):
                            nc.tensor.matmul(
                                ps[:, 512 * qh : 512 * (qh + 1)],
                                lhs,
                                xr[:, N * ct + 512 * qh : N * ct + 512 * (qh + 1)],
                                start=(ct == 0),
                                stop=(ct == NCT - 1),
                            )
                    qk_evac(ft, ps)

                filler = collections.deque()

                def queue_qk_chunks(ft, wqs):
                    """qkT block ft as 6 filler chunks (2 matmuls each),
                    accumulating in a u-tag PSUM slot."""
                    cell = {}

                    def chunk(ct):
                        if ct == 0:
                            cell["ps"] = psp.tile(
                                [128, N], F32, tag="u", name="qk_acc"
                            )
                        ps = cell["ps"]
                        lhs = wqs[:, 128 * ct : 128 * (ct + 1)]
                        for qh in range(2):
                            nc.tensor.matmul(
                                ps[:, 512 * qh : 512 * (qh + 1)],
                                lhs,
                                xr[:, N * ct + 512 * qh : N * ct + 512 * (qh + 1)],
                                start=(ct == 0),
                                stop=(ct == NCT - 1),
                            )
                        if ct == NCT - 1:
                            qk_evac(ft, ps)

                    for ct in range(NCT):
                        filler.append(lambda ct=ct: chunk(ct))

                def v_block(m, tag="s"):
                    """v token-tile m -> vp [128, 780]: 12x(64 v cols + ones)."""
                    ps = psp.tile([128, N], F32, tag=tag, name="v_acc")
                    for ct in range(NCT):
                        lhs = xr[:, N * ct + 128 * m : N * ct + 128 * (m + 1)]
                        for nn, nw in ((0, 512), (512, 256)):
                            nc.tensor.matmul(
                                ps[:, nn : nn + nw],
                                lhs,
                                wv_t[:, C * ct + nn : C * ct + nn + nw],
                                start=(ct == 0),
                                stop=(ct == NCT - 1),
                            )
                    blk = vp_t[:, WV * m : WV * (m + 1)].rearrange(
                        "p (h c) -> p h c", c=HD + 1
                    )
                    nc.vector.tensor_copy(
                        blk[:, :, 0:HD],
                        ps[:, 0:C].rearrange("p (h c) -> p h c", c=HD),
                    )
                    nc.vector.tensor_copy(
                        blk[:, :, HD : HD + 1],
                        ones12[:].rearrange("p (h o) -> p h o", o=1),
                    )

                def head0_split():
                    """Head 0 in two waves of 4 k-tiles: scores+exp emitted
                    before that wave's v blocks, so ACT drains exps while the
                    PE computes v. Wave size matches pt_bufs."""
                    qft, po = 0, 0
                    wave = min(pt_bufs, 4)
                    ps_u = psp.tile([128, N], F32, tag="u")
                    for w0 in range(0, NTT, wave):
                        pts = []
                        for kt in range(w0, w0 + wave):
                            ps_s = psp.tile([128, N], F32, tag="s")
                            ksl = qk_k[:, 128 * kt : 128 * (kt + 1)]
                            for qh in range(2):
                                nc.tensor.matmul(
                                    ps_s[:, 512 * qh : 512 * (qh + 1)],
                                    ksl,
                                    qk_q[
                                        :,
                                        N * qft + 512 * qh : N * qft + 512 * (qh + 1),
                                    ],
                                    start=True,
                                    stop=True,
                                )
                            pt = ptp.tile([128, N], BF16, tag="pt")
                            nc.scalar.activation(pt[:], ps_s[:], EXP)
                            pts.append(pt)
                        for m in range(w0, w0 + wave):
                            v_block(m)
                        for kt in range(w0, w0 + wave):
                            vsl = vp_t[:, WV * kt : WV * kt + HD + 1]
                            for qh in range(2):
                                sl = slice(512 * qh, 512 * (qh + 1))
                                nc.tensor.matmul(
                                    ps_u[0:65, sl], vsl, pts[kt - w0][:, sl],
                                    start=(kt == 0), stop=(kt == NTT - 1),
                                )
                            if filler:
                                filler.popleft()()
                    uT = wkp.tile([128, N], F32, tag="uT", bufs=1)
                    nc.vector.tensor_copy(uT[0:65, :], ps_u[0:65, :])
                    rec_f = wkp.tile([1, N], F32, tag="recf2", bufs=1)
                    nc.vector.reciprocal(rec_f[:], uT[64:65, :])
                    bc = wkp.tile([64, N], F32, tag="bc", bufs=1)
                    nc.gpsimd.partition_broadcast(bc[:], rec_f[:])
                    nc.vector.tensor_tensor(
                        attnT[po : po + 64, N * qft : N * (qft + 1)],
                        uT[0:64, :],
                        bc[:],
                        op=MULT,
                    )

                def head(h):
                    qft, po = h // 2, 64 * (h % 2)
                    ps_u = psp.tile([128, N], F32, tag="u")

                    def attnv(kt, pt):
                        vsl = vp_t[
                            :, WV * kt + (HD + 1) * h : WV * kt + (HD + 1) * (h + 1)
                        ]
                        for qh in range(2):
                            sl = slice(512 * qh, 512 * (qh + 1))
                            nc.tensor.matmul(
                                ps_u[0:65, sl], vsl, pt[:, sl],
                                start=(kt == 0), stop=(kt == NTT - 1),
                            )

                    # attn@v lags the exps by one k-tile so its pt input is
                    # long-ready (no sem-gated PE stall / LDW prefetch break)
                    prev = None
                    for kt in range(NTT):
                        ps_s = psp.tile([128, N], F32, tag="s")
                        ksl = qk_k[
                            :, N * h + 128 * kt : N * h + 128 * (kt + 1)
                        ]
                        for qh in range(2):
                            nc.tensor.matmul(
                                ps_s[:, 512 * qh : 512 * (qh + 1)],
                                ksl,
                                qk_q[
                                    :,
                                    N * qft + 512 * qh : N * qft + 512 * (qh + 1),
                                ],
                                start=True,
                                stop=True,
                            )
                        pt = ptp.tile([128, N], BF16, tag="pt")
                        nc.scalar.activation(pt[:], ps_s[:], EXP)
                        if prev is not None:
                            attnv(kt - 1, prev)
                        prev = pt
                        if filler and (kt >= 2 or h % 2 == 1):
                            filler.popleft()()
                    attnv(NTT - 1, prev)
                    # evacuate U+den, normalize off the PE:
                    # partition_broadcast den (gpsimd) -> divide (DVE)
                    uT = wkp.tile([128, N], F32, tag="uT", bufs=1)
                    nc.vector.tensor_copy(uT[0:65, :], ps_u[0:65, :])
                    rec_f = wkp.tile([1, N], F32, tag="recf2", bufs=1)
                    nc.vector.reciprocal(rec_f[:], uT[64:65, :])
                    bc = wkp.tile([64, N], F32, tag="bc", bufs=1)
                    nc.gpsimd.partition_broadcast(bc[:], rec_f[:])
                    nc.vector.tensor_tensor(
                        attnT[po : po + 64, N * qft : N * (qft + 1)],
                        uT[0:64, :],
                        bc[:],
                        op=MULT,
                    )

                # pre-head phase: blocks 0,6; head 0 split (v inside)
                qk_compute(0, w_first)
                qk_compute(6, w_second)

                # heads with deadline-scheduled qk fillers:
                # pair t (blocks t, 6+t) loads at head 2t-3, chunks during
                # heads 2t-2 / 2t-1, needed by head 2t.
                loads = {}
                loads[0] = (wq_load(1), wq_load(7))  # before head 0
                for h in range(H):
                    t = h // 2 + 1
                    if h % 2 == 0 and t <= 5:
                        wa, wb = loads.pop(h)
                        queue_qk_chunks(t, wa)
                        queue_qk_chunks(6 + t, wb)
                        if t + 1 <= 5:
                            loads[h + 2] = (wq_load(t + 1), wq_load(7 + t))
                    if h == 6:
                        nc.sync.dma_start(
                            wp_t[:].rearrange("p (ct f) -> p ct f", f=C),
                            wp_d[:].rearrange("(ct p) f -> p ct f", p=128),
                        )
                    if h == 0:
                        head0_split()
                    else:
                        head(h)
                while filler:
                    filler.popleft()()

                # projection
                for m in range(NTT):
                    ps_o = psp.tile([128, N], F32, tag="s")
                    for ct in range(NCT - 1):
                        lhs = attnT[:, N * ct + 128 * m : N * ct + 128 * (m + 1)]
                        for nn, nw in ((0, 512), (512, 256)):
                            nc.tensor.matmul(
                                ps_o[:, nn : nn + nw],
                                lhs,
                                wp_t[:, C * ct + nn : C * ct + nn + nw],
                                start=(ct == 0),
                                stop=False,
                            )
                    ct = NCT - 1
                    lhs = attnT[:, N * ct + 128 * m : N * ct + 128 * (m + 1)]
                    for nn, nw in ((0, 512), (512, 256)):
                        nc.tensor.matmul(
                            ps_o[:, nn : nn + nw],
                            lhs,
                            wp_t[:, C * ct + nn : C * ct + nn + nw],
                            start=False,
                            stop=True,
                        )
                    o_sb = wkp.tile([128, C], F32, tag="osb", bufs=2)
                    nc.vector.tensor_tensor(
                        o_sb[:], ps_o[:, 0:C], bias_sb[:], op=mybir.AluOpType.add
                    )
                    # out on the ACT hwdge queue: keeps 8.8us of f32 output
                    # drain off the SP ring so the next rep's input loads
                    # (xr/wv/wp on SP) start immediately at the boundary
                    nc.scalar.dma_start(out_d[128 * m : 128 * (m + 1), :], o_sb[:])

            warmup()
            if reps:
                with tc.For_i(0, reps, 1):
                    emit()
            else:
                emit()

    nc.compile()
    return nc


_CACHE = {}


def _get_nc():
    if "nc" not in _CACHE:
        _CACHE["nc"] = _build()
    return _CACHE["nc"]


def _host_prep(w_qkv, w_proj, b_proj):
    import ml_dtypes

    bf = ml_dtypes.bfloat16
    ws = np.asarray(w_qkv, dtype=np.float32).copy()
    ws[0:C] *= SCALE
    wt = np.ascontiguousarray(ws.T)  # [768, 2304]
    # [12, 128, 768]: wqb[ft, p, ct*128+f] = wt[ct*128+p, ft*128+f] so each
    # per-block DMA lands as 128 contiguous 1536B partition lines
    wqb = np.ascontiguousarray(
        wt[:, : 2 * C]
        .reshape(NCT, 128, NQK, 128)
        .transpose(2, 1, 0, 3)
        .reshape(NQK, 128, C)
    ).astype(bf)
    wv = np.ascontiguousarray(wt[:, 2 * C :]).astype(bf)
    wp = np.ascontiguousarray(np.asarray(w_proj, dtype=np.float32).T).astype(bf)
    bp = np.ascontiguousarray(
        np.tile(np.asarray(b_proj, dtype=np.float32)[None, :], (128, 1))
    )
    return wqb, wv, wp, bp


def _prep_xT(xb):
    import ml_dtypes

    return np.ascontiguousarray(xb.T).astype(ml_dtypes.bfloat16)


def kernel(x, w_qkv, w_proj, b_proj):
    x = np.asarray(x, dtype=np.float32)
    assert x.shape == (B, N, C), x.shape
    wqb, wv, wp, bp = _host_prep(w_qkv, w_proj, b_proj)
    in_maps = [
        {
            "xT": _prep_xT(x[b]),
            "wqb": wqb,
            "wv": wv,
            "wp": wp,
            "bp": bp,
        }
        for b in range(B)
    ]
    nc = _get_nc()
    res = bass_utils.run_bass_kernel_spmd(nc, in_maps, core_ids=list(range(B)))
    return np.stack([np.asarray(res.results[b]["out"]) for b in range(B)]).astype(
        np.float32
    )



# revision 29
# speedup vs baseline: 1.0177x; 1.0177x over previous
"""Trainium2 Bass kernel for nn_Attention_9998683865539.

Multi-head attention (B=8, N=1024, C=768, H=12, HD=64, fp32), data-parallel
over the batch across 8 NeuronCores (one batch element per core, weights
replicated, no collectives). All matmul operands are bf16 (inputs are
pre-cast on the host, halving HBM traffic); PSUM accumulation is fp32, so
end-to-end error vs the fp32 reference is ~6e-3 relative (gate: 2e-2).

Per-core dataflow:
  qkT  = (w_qkv_scaled.T).T @ xT    feature-major; q-rows of w_qkv are
                                    pre-scaled by HD^-0.5 on the host.
                                    q blocks stay packed (2 heads per 128
                                    partitions); k blocks are unpacked into
                                    per-head [128, N] bf16 tiles whose
                                    complementary 64 partitions are EXACTLY
                                    ZERO (one-time memset outside the loop).
  v    = x @ w_v.T                  token-major, packed per head with a
                                    trailing ones column (V' = [v | 1])
  per head h:
    S.T[k, q] = k_pad_h.T @ q_packed    K=128 full-array matmul: the zero
                                    rows of k select just head h while the
                                    partner head's q rows contribute 0.
                                    (A K=64 matmul runs at HALF rate on the
                                    128x128 PE unless row-tiled, and mode
                                    switches between 64/128-row tiling cost
                                    ~0.5us, so zero-padding to K=128 beats
                                    both alternatives in this schedule.)
    P.T = exp(S.T)                  ScalarE, no max-subtraction (|S|<~7);
                                    attn@v lags exps by one k-tile so its
                                    pt input is ready when the PE arrives
    [U.T; den] = V'.T @ P.T         M=65 matmul, PSUM accum over k-tiles;
                                    row 64 is the softmax denominator
    attnT_h = U.T * bcast(1/den)    DVE reciprocal + GPSIMD
                                    partition_broadcast + DVE multiply
  out = attnT.T @ w_proj.T + b_proj    bias added on DVE, out streamed f32

qk feature blocks 1..5, 7..11 are emitted as "filler" matmul chunks spliced
between head k-tile iterations so the PE stays busy while ACT drains exps.
One-time work (ACT exp-table warm, PE spin-up, constants, qk_k zeroing,
bias load) is hoisted out of the For_i repeat loop.
"""
import sys

sys.path.insert(0, "/opt/trn_rl_repo")

import collections

import numpy as np

import concourse.bass as bass
import concourse.tile as tile
from concourse import bacc, mybir
from concourse import bass_utils

F32 = mybir.dt.float32
F32R = mybir.dt.float32r
BF16 = mybir.dt.bfloat16
EXP = mybir.ActivationFunctionType.Exp
MULT = mybir.AluOpType.mult

B = 8            # batch (one element per core)
C = 768          # channels
N = 1024         # tokens
H = 12           # heads
HD = 64          # head dim
SCALE = HD ** -0.5
NCT = C // 128   # 6 channel tiles
NTT = N // 128   # 8 token tiles
NQK = 12         # qk feature tiles (1536/128)
WV = H * (HD + 1)  # 780: per token-tile, 12 heads x (64 v + 1 ones)


def _build(reps=0, pt_bufs=6, wqs_bufs=4):
    nc = bacc.Bacc("TRN2", target_bir_lowering=False, debug=False)

    xT_d = nc.dram_tensor("xT", [C, N], BF16, kind="ExternalInput").ap()
    wqb_d = nc.dram_tensor("wqb", [NQK, C, 128], BF16, kind="ExternalInput").ap()
    wv_d = nc.dram_tensor("wv", [C, C], BF16, kind="ExternalInput").ap()
    wp_d = nc.dram_tensor("wp", [C, C], BF16, kind="ExternalInput").ap()
    bp_d = nc.dram_tensor("bp", [128, C], F32, kind="ExternalInput").ap()
    out_d = nc.dram_tensor("out", [N, C], F32, kind="ExternalOutput").ap()

    with tile.TileContext(nc) as tc:
        with (
            tc.tile_pool(name="big", bufs=1) as big,
            tc.tile_pool(name="ptp", bufs=pt_bufs) as ptp,
            tc.tile_pool(name="wkp", bufs=1) as wkp,
            tc.tile_pool(name="psp", bufs=2, space=bass.MemorySpace.PSUM) as psp,
        ):
            qk_q = big.tile([128, NCT * N], BF16)     # 12KB/part: q blocks, packed
            qk_k = big.tile([128, H * N], BF16)       # 24KB/part: per-head k,
            #   complementary 64 partitions exactly zero so K=128 matmuls
            #   against packed q blocks select just this head's features
            vp_t = big.tile([128, NTT * WV], BF16)    # 12.2KB/part
            attnT = big.tile([128, NCT * N], BF16)    # 12KB/part
            xr = big.tile([128, NCT * N], BF16)       # 12KB/part
            wv_t = big.tile([128, NCT * C], BF16)     # 9KB/part
            wp_t = big.tile([128, NCT * C], BF16)     # 9KB/part
            ones12 = wkp.tile([128, H], F32)
            bias_sb = wkp.tile([128, C], F32)
            warm = wkp.tile([128, 1], F32)

            def warmup():
                # one-time: constants, ACT exp-table load, PE spin-up, bias
                nc.vector.memset(ones12[:], 1.0)
                nc.scalar.activation(warm[:], ones12[:, 0:1], EXP)
                ps_w = psp.tile([128, N], F32, tag="s", name="ps_warm")
                for _ in range(30):
                    nc.tensor.matmul(
                        ps_w[0:H, 0:H], ones12[:], ones12[:],
                        start=True, stop=True,
                    )
                nc.sync.dma_start(bias_sb[:], bp_d[:])
                # zero qk_k so each head's complementary 64 partitions stay
                # exactly zero (in-loop evacuations only write the data half)
                nc.vector.memset(qk_k[:], 0.0)

            def emit():
                def wq_load(ft):
                    wqs = wkp.tile(
                        [128, NCT * 128], BF16, tag="wqs", bufs=wqs_bufs
                    )
                    nc.gpsimd.dma_start(
                        wqs[:].rearrange("p (ct f) -> p ct f", f=128),
                        wqb_d[ft].rearrange("(ct p) f -> p ct f", p=128),
                    )
                    return wqs

                # DMA order: wq block 0 (per-ct chunks interleaved with xT
                # chunks) first, then block 6, then wv; wp + rest trail.
                w_first = wkp.tile([128, NCT * 128], BF16, tag="wqs", bufs=wqs_bufs, name="w_first")
                for ct in range(NCT):
                    nc.gpsimd.dma_start(
                        w_first[:, 128 * ct : 128 * (ct + 1)],
                        wqb_d[0][128 * ct : 128 * (ct + 1), :],
                    )
                    nc.sync.dma_start(
                        xr[:, N * ct : N * (ct + 1)],
                        xT_d[128 * ct : 128 * (ct + 1), :],
                    )
                w_second = wq_load(6)
                nc.sync.dma_start(
                    wv_t[:].rearrange("p (ct f) -> p ct f", f=C),
                    wv_d[:].rearrange("(ct p) f -> p ct f", p=128),
                )

                def qk_evac(ft, ps):
                    if ft < NCT:  # q block: packed bf16
                        nc.vector.tensor_copy(
                            qk_q[:, N * ft : N * (ft + 1)], ps[:]
                        )
                    else:  # k block: unpack into per-head zero-padded blocks
                        t = ft - NCT
                        nc.vector.tensor_copy(
                            qk_k[0:64, N * 2 * t : N * (2 * t + 1)], ps[0:64, :]
                        )
                        nc.vector.tensor_copy(
                            qk_k[64:128, N * (2 * t + 1) : N * (2 * t + 2)],
                            ps[64:128, :],
                        )

                def qk_compute(ft, wqs):
                    """qkT block ft, monolithic (pre-head phase)."""
                    ps = psp.tile([128, N], F32, tag="s")
                    for ct in range(NCT):
                        lhs = wqs[:, 128 * ct : 128 * (ct + 1)]
                        for qh in range(2):
                            nc.tensor.matmul(
                                ps[:, 512 * qh : 512 * (qh + 1)],
                                lhs,
                                xr[:, N * ct + 512 * qh : N * ct + 512 * (qh + 1)],
                                start=(ct == 0),
                                stop=(ct == NCT - 1),
                            )
                    qk_evac(ft, ps)

                filler = collections.deque()

                def queue_qk_chunks(ft, wqs):
                    """qkT block ft as 6 filler chunks (2 matmuls each),
                    accumulating in a u-tag PSUM slot."""
                    cell = {}

                    def chunk(ct):
                        if ct == 0:
                            cell["ps"] = psp.tile(
                                [128, N], F32, tag="u", name="qk_acc"
                            )
                        ps = cell["ps"]
                        lhs = wqs[:, 128 * ct : 128 * (ct + 1)]
                        for qh in range(2):
                            nc.tensor.matmul(
                                ps[:, 512 * qh : 512 * (qh + 1)],
                                lhs,
                                xr[:, N * ct + 512 * qh : N * ct + 512 * (qh + 1)],
                                start=(ct == 0),
                                stop=(ct == NCT - 1),
                            )
                        if ct == NCT - 1:
                            qk_evac(ft, ps)

                    for ct in range(NCT):
                        filler.append(lambda ct=ct: chunk(ct))

                def v_block(m, tag="s"):
                    """v token-tile m -> vp [128, 780]: 12x(64 v cols + ones)."""
                    ps = psp.tile([128, N], F32, tag=tag, name="v_acc")
                    for ct in range(NCT):
                        lhs = xr[:, N * ct + 128 * m : N * ct + 128 * (m + 1)]
                        for nn, nw in ((0, 512), (512, 256)):
                            nc.tensor.matmul(
                                ps[:, nn : nn + nw],
                                lhs,
                                wv_t[:, C * ct + nn : C * ct + nn + nw],
                                start=(ct == 0),
                                stop=(ct == NCT - 1),
                            )
                    blk = vp_t[:, WV * m : WV * (m + 1)].rearrange(
                        "p (h c) -> p h c", c=HD + 1
                    )
                    nc.vector.tensor_copy(
                        blk[:, :, 0:HD],
                        ps[:, 0:C].rearrange("p (h c) -> p h c", c=HD),
                    )
                    nc.vector.tensor_copy(
                        blk[:, :, HD : HD + 1],
                        ones12[:].rearrange("p (h o) -> p h o", o=1),
                    )

                def head0_split():
                    """Head 0 in two waves of 4 k-tiles: scores+exp emitted
                    before that wave's v blocks, so ACT drains exps while the
                    PE computes v. Wave size matches pt_bufs."""
                    qft, po = 0, 0
                    wave = min(pt_bufs, 4)
                    ps_u = psp.tile([128, N], F32, tag="u")
                    for w0 in range(0, NTT, wave):
                        pts = []
                        for kt in range(w0, w0 + wave):
                            ps_s = psp.tile([128, N], F32, tag="s")
                            ksl = qk_k[:, 128 * kt : 128 * (kt + 1)]
                            for qh in range(2):
                                nc.tensor.matmul(
                                    ps_s[:, 512 * qh : 512 * (qh + 1)],
                                    ksl,
                                    qk_q[
                                        :,
                                        N * qft + 512 * qh : N * qft + 512 * (qh + 1),
                                    ],
                                    start=True,
                                    stop=True,
                                )
                            pt = ptp.tile([128, N], BF16, tag="pt")
                            nc.scalar.activation(pt[:], ps_s[:], EXP)
                            pts.append(pt)
                        for m in range(w0, w0 + wave):
                            v_block(m)
                        for kt in range(w0, w0 + wave):
                            vsl = vp_t[:, WV * kt : WV * kt + HD + 1]
                            for qh in range(2):
                                sl = slice(512 * qh, 512 * (qh + 1))
                                nc.tensor.matmul(
                                    ps_u[0:65, sl], vsl, pts[kt - w0][:, sl],
                                    start=(kt == 0), stop=(kt == NTT - 1),
                                )
                            if filler:
                                filler.popleft()()
                    uT = wkp.tile([128, N], F32, tag="uT", bufs=1)
                    nc.vector.tensor_copy(uT[0:65, :], ps_u[0:65, :])
                    rec_f = wkp.tile([1, N], F32, tag="recf2", bufs=1)
                    nc.vector.reciprocal(rec_f[:], uT[64:65, :])
                    bc = wkp.tile([64, N], F32, tag="bc", bufs=1)
                    nc.gpsimd.partition_broadcast(bc[:], rec_f[:])
                    nc.vector.tensor_tensor(
                        attnT[po : po + 64, N * qft : N * (qft + 1)],
                        uT[0:64, :],
                        bc[:],
                        op=MULT,
                    )

                def head(h):
                    qft, po = h // 2, 64 * (h % 2)
                    ps_u = psp.tile([128, N], F32, tag="u")

                    def attnv(kt, pt):
                        vsl = vp_t[
                            :, WV * kt + (HD + 1) * h : WV * kt + (HD + 1) * (h + 1)
                        ]
                        for qh in range(2):
                            sl = slice(512 * qh, 512 * (qh + 1))
                            nc.tensor.matmul(
                                ps_u[0:65, sl], vsl, pt[:, sl],
                                start=(kt == 0), stop=(kt == NTT - 1),
                            )

                    # attn@v lags the exps by one k-tile so its pt input is
                    # long-ready (no sem-gated PE stall / LDW prefetch break)
                    prev = None
                    for kt in range(NTT):
                        ps_s = psp.tile([128, N], F32, tag="s")
                        ksl = qk_k[
                            :, N * h + 128 * kt : N * h + 128 * (kt + 1)
                        ]
                        for qh in range(2):
                            nc.tensor.matmul(
                                ps_s[:, 512 * qh : 512 * (qh + 1)],
                                ksl,
                                qk_q[
                                    :,
                                    N * qft + 512 * qh : N * qft + 512 * (qh + 1),
                                ],
                                start=True,
                                stop=True,
                            )
                        pt = ptp.tile([128, N], BF16, tag="pt")
                        nc.scalar.activation(pt[:], ps_s[:], EXP)
                        if prev is not None:
                            attnv(kt - 1, prev)
                        prev = pt
                        if filler and (kt >= 2 or h % 2 == 1):
                            filler.popleft()()
                    attnv(NTT - 1, prev)
                    # evacuate U+den, normalize off the PE:
                    # partition_broadcast den (gpsimd) -> divide (DVE)
                    uT = wkp.tile([128, N], F32, tag="uT", bufs=1)
                    nc.vector.tensor_copy(uT[0:65, :], ps_u[0:65, :])
                    rec_f = wkp.tile([1, N], F32, tag="recf2", bufs=1)
                    nc.vector.reciprocal(rec_f[:], uT[64:65, :])
                    bc = wkp.tile([64, N], F32, tag="bc", bufs=1)
                    nc.gpsimd.partition_broadcast(bc[:], rec_f[:])
                    nc.vector.tensor_tensor(
                        attnT[po : po + 64, N * qft : N * (qft + 1)],
                        uT[0:64, :],
                        bc[:],
                        op=MULT,
                    )

                # pre-head phase: blocks 0,6; head 0 split (v inside)
                qk_compute(0, w_first)
                qk_compute(6, w_second)

                # heads with deadline-scheduled qk fillers:
                # pair t (blocks t, 6+t) loads at head 2t-3, chunks during
                # heads 2t-2 / 2t-1, needed by head 2t.
                loads = {}
                loads[0] = (wq_load(1), wq_load(7))  # before head 0
                for h in range(H):
                    t = h // 2 + 1
                    if h % 2 == 0 and t <= 5:
                        wa, wb = loads.pop(h)
                        queue_qk_chunks(t, wa)
                        queue_qk_chunks(6 + t, wb)
                        if t + 1 <= 5:
                            loads[h + 2] = (wq_load(t + 1), wq_load(7 + t))
                    if h == 6:
                        nc.sync.dma_start(
                            wp_t[:].rearrange("p (ct f) -> p ct f", f=C),
                            wp_d[:].rearrange("(ct p) f -> p ct f", p=128),
                        )
                    if h == 0:
                        head0_split()
                    else:
                        head(h)
                while filler:
                    filler.popleft()()

                # projection
                for m in range(NTT):
                    ps_o = psp.tile([128, N], F32, tag="s")
                    for ct in range(NCT - 1):
                        lhs = attnT[:, N * ct + 128 * m : N * ct + 128 * (m + 1)]
                        for nn, nw in ((0, 512), (512, 256)):
                            nc.tensor.matmul(
                                ps_o[:, nn : nn + nw],
                                lhs,
                                wp_t[:, C * ct + nn : C * ct + nn + nw],
                                start=(ct == 0),
                                stop=False,
                            )
                    ct = NCT - 1
                    lhs = attnT[:, N * ct + 128 * m : N * ct + 128 * (m + 1)]
                    for nn, nw in ((0, 512), (512, 256)):
                        nc.tensor.matmul(
                            ps_o[:, nn : nn + nw],
                            lhs,
                            wp_t[:, C * ct + nn : C * ct + nn + nw],
                            start=False,
                            stop=True,
                        )
                    o_sb = wkp.tile([128, C], F32, tag="osb", bufs=2)
                    nc.vector.tensor_tensor(
                        o_sb[:], ps_o[:, 0:C], bias_sb[:], op=mybir.AluOpType.add
                    )
                    nc.sync.dma_start(out_d[128 * m : 128 * (m + 1), :], o_sb[:])

            warmup()
            if reps:
                with tc.For_i(0, reps, 1):
                    emit()
            else:
                emit()

    nc.compile()
    return nc


_CACHE = {}


def _get_nc():
    if "nc" not in _CACHE:
        _CACHE["nc"] = _build()
    return _CACHE["nc"]


def _host_prep(w_qkv, w_proj, b_proj):
    import ml_dtypes

    bf = ml_dtypes.bfloat16
    ws = np.asarray(w_qkv, dtype=np.float32).copy()
    ws[0:C] *= SCALE
    wt = np.ascontiguousarray(ws.T)  # [768, 2304]
    wqb = np.ascontiguousarray(
        wt[:, : 2 * C].reshape(C, NQK, 128).transpose(1, 0, 2)
    ).astype(bf)
    wv = np.ascontiguousarray(wt[:, 2 * C :]).astype(bf)
    wp = np.ascontiguousarray(np.asarray(w_proj, dtype=np.float32).T).astype(bf)
    bp = np.ascontiguousarray(
        np.tile(np.asarray(b_proj, dtype=np.float32)[None, :], (128, 1))
    )
    return wqb, wv, wp, bp


def _prep_xT(xb):
    import ml_dtypes

    return np.ascontiguousarray(xb.T).astype(ml_dtypes.bfloat16)


def kernel(x, w_qkv, w_proj, b_proj):
    x = np.asarray(x, dtype=np.float32)
    assert x.shape == (B, N, C), x.shape
    wqb, wv, wp, bp = _host_prep(w_qkv, w_proj, b_proj)
    in_maps = [
        {
            "xT": _prep_xT(x[b]),
            "wqb": wqb,
            "wv": wv,
            "wp": wp,
            "bp": bp,
        }
        for b in range(B)
    ]
    nc = _get_nc()
    res = bass_utils.run_bass_kernel_spmd(nc, in_maps, core_ids=list(range(B)))
    return np.stack([np.asarray(res.results[b]["out"]) for b in range(B)]).astype(
        np.float32
    )



# revision 31
# speedup vs baseline: 1.2680x; 1.2459x over previous
"""Trainium2 Bass kernel for nn_Attention_9998683865539.

Multi-head attention (B=8, N=1024, C=768, H=12, HD=64, fp32), data-parallel
over the batch across 8 NeuronCores (one batch element per core, weights
replicated, no collectives). All matmul operands are bf16 (inputs are
pre-cast on the host, halving HBM traffic); PSUM accumulation is fp32, so
end-to-end error vs the fp32 reference is ~6e-3 relative (gate: 2e-2).

Per-core dataflow:
  qkT  = (w_qkv_scaled.T).T @ xT    feature-major; q-rows of w_qkv are
                                    pre-scaled by HD^-0.5 on the host.
                                    q blocks stay packed (2 heads per 128
                                    partitions); k blocks are unpacked into
                                    per-head [128, N] bf16 tiles whose
                                    complementary 64 partitions are EXACTLY
                                    ZERO (one-time memset outside the loop).
  v    = x @ w_v.T                  token-major, packed per head with a
                                    trailing ones column (V' = [v | 1])
  per head h:
    S.T[k, q] = k_pad_h.T @ q_packed    K=128 full-array matmul: the zero
                                    rows of k select just head h while the
                                    partner head's q rows contribute 0.
                                    (A K=64 matmul runs at HALF rate on the
                                    128x128 PE unless row-tiled, and mode
                                    switches between 64/128-row tiling cost
                                    ~0.5us, so zero-padding to K=128 beats
                                    both alternatives in this schedule.)
    P.T = exp(S.T)                  ScalarE, no max-subtraction (|S|<~7);
                                    attn@v lags exps by one k-tile so its
                                    pt input is ready when the PE arrives
    [U.T; den] = V'.T @ P.T         M=65 matmul, PSUM accum over k-tiles;
                                    row 64 is the softmax denominator
    attnT_h = U.T * bcast(1/den)    DVE reciprocal + GPSIMD
                                    partition_broadcast + DVE multiply
  out = attnT.T @ w_proj.T + b_proj    bias added on DVE, out streamed f32

qk feature blocks 1..5, 7..11 are emitted as "filler" matmul chunks spliced
between head k-tile iterations so the PE stays busy while ACT drains exps.
One-time work (ACT exp-table warm, PE spin-up, constants, qk_k zeroing,
bias load) is hoisted out of the For_i repeat loop.

The repeat loop runs two emit() bodies per iteration with ping-pong x/wv/wp
input sets: emit(par) computes from set [par] while prefetching the next
emit's inputs into set [1-par], so each rep starts with inputs resident and
the 8.8us f32 output drain on the SP queue never blocks the next rep's
loads (this removed a ~7us PE stall at every rep boundary, found via a
2-rep CoreSim trace).
"""
import sys

sys.path.insert(0, "/opt/trn_rl_repo")

import collections

import numpy as np

import concourse.bass as bass
import concourse.tile as tile
from concourse import bacc, mybir
from concourse import bass_utils

F32 = mybir.dt.float32
F32R = mybir.dt.float32r
BF16 = mybir.dt.bfloat16
EXP = mybir.ActivationFunctionType.Exp
MULT = mybir.AluOpType.mult

B = 8            # batch (one element per core)
C = 768          # channels
N = 1024         # tokens
H = 12           # heads
HD = 64          # head dim
SCALE = HD ** -0.5
NCT = C // 128   # 6 channel tiles
NTT = N // 128   # 8 token tiles
NQK = 12         # qk feature tiles (1536/128)
WV = H * (HD + 1)  # 780: per token-tile, 12 heads x (64 v + 1 ones)


def _build(reps=0, pt_bufs=6, wqs_bufs=4):
    nc = bacc.Bacc("TRN2", target_bir_lowering=False, debug=False)

    xT_d = nc.dram_tensor("xT", [C, N], BF16, kind="ExternalInput").ap()
    wqb_d = nc.dram_tensor("wqb", [NQK, C, 128], BF16, kind="ExternalInput").ap()
    wv_d = nc.dram_tensor("wv", [C, C], BF16, kind="ExternalInput").ap()
    wp_d = nc.dram_tensor("wp", [C, C], BF16, kind="ExternalInput").ap()
    bp_d = nc.dram_tensor("bp", [128, C], F32, kind="ExternalInput").ap()
    out_d = nc.dram_tensor("out", [N, C], F32, kind="ExternalOutput").ap()

    with tile.TileContext(nc) as tc:
        with (
            tc.tile_pool(name="big", bufs=1) as big,
            tc.tile_pool(name="ptp", bufs=pt_bufs) as ptp,
            tc.tile_pool(name="wkp", bufs=1) as wkp,
            tc.tile_pool(name="psp", bufs=2, space=bass.MemorySpace.PSUM) as psp,
        ):
            qk_q = big.tile([128, NCT * N], BF16)     # 12KB/part: q blocks, packed
            qk_k = big.tile([128, H * N], BF16)       # 24KB/part: per-head k,
            #   complementary 64 partitions exactly zero so K=128 matmuls
            #   against packed q blocks select just this head's features
            vp_t = big.tile([128, NTT * WV], BF16)    # 12.2KB/part
            attnT = big.tile([128, NCT * N], BF16)    # 12KB/part
            # ping-pong input sets: emit(par) computes from set [par] while
            # issuing the NEXT emit's loads into set [1-par], so every rep
            # starts with its inputs already resident (no boundary stall)
            xrs = [
                big.tile([128, NCT * N], BF16, name=f"xr{i}") for i in (0, 1)
            ]
            wvs = [
                big.tile([128, NCT * C], BF16, name=f"wv{i}") for i in (0, 1)
            ]
            wps = [
                big.tile([128, NCT * C], BF16, name=f"wp{i}") for i in (0, 1)
            ]
            ones12 = wkp.tile([128, H], F32)
            bias_sb = wkp.tile([128, C], F32)
            warm = wkp.tile([128, 1], F32)

            def warmup():
                # one-time: constants, ACT exp-table load, PE spin-up, bias
                nc.vector.memset(ones12[:], 1.0)
                nc.scalar.activation(warm[:], ones12[:, 0:1], EXP)
                ps_w = psp.tile([128, N], F32, tag="s", name="ps_warm")
                for _ in range(30):
                    nc.tensor.matmul(
                        ps_w[0:H, 0:H], ones12[:], ones12[:],
                        start=True, stop=True,
                    )
                nc.sync.dma_start(bias_sb[:], bp_d[:])
                load_inputs(0)
                nc.sync.dma_start(
                    wps[0][:].rearrange("p (ct f) -> p ct f", f=C),
                    wp_d[:].rearrange("(ct p) f -> p ct f", p=128),
                )
                # zero qk_k so each head's complementary 64 partitions stay
                # exactly zero (in-loop evacuations only write the data half)
                nc.vector.memset(qk_k[:], 0.0)

            def load_inputs(nxt):
                for ct in range(NCT):
                    nc.sync.dma_start(
                        xrs[nxt][:, N * ct : N * (ct + 1)],
                        xT_d[128 * ct : 128 * (ct + 1), :],
                    )
                nc.sync.dma_start(
                    wvs[nxt][:].rearrange("p (ct f) -> p ct f", f=C),
                    wv_d[:].rearrange("(ct p) f -> p ct f", p=128),
                )

            def emit(par):
                xr = xrs[par]
                wv_t = wvs[par]
                wp_t = wps[par]

                def wq_load(ft):
                    wqs = wkp.tile(
                        [128, NCT * 128], BF16, tag="wqs", bufs=wqs_bufs
                    )
                    nc.gpsimd.dma_start(
                        wqs[:].rearrange("p (ct f) -> p ct f", f=128),
                        wqb_d[ft].rearrange("(ct p) f -> p ct f", p=128),
                    )
                    return wqs

                w_first = wkp.tile([128, NCT * 128], BF16, tag="wqs", bufs=wqs_bufs, name="w_first")
                for ct in range(NCT):
                    nc.gpsimd.dma_start(
                        w_first[:, 128 * ct : 128 * (ct + 1)],
                        wqb_d[0][128 * ct : 128 * (ct + 1), :],
                    )
                w_second = wq_load(6)
                # prefetch the NEXT emit's x and wv (this emit's inputs are
                # already resident, loaded by the previous emit / warmup)
                load_inputs(1 - par)

                def qk_evac(ft, ps):
                    if ft < NCT:  # q block: packed bf16
                        nc.vector.tensor_copy(
                            qk_q[:, N * ft : N * (ft + 1)], ps[:]
                        )
                    else:  # k block: unpack into per-head zero-padded blocks
                        t = ft - NCT
                        nc.vector.tensor_copy(
                            qk_k[0:64, N * 2 * t : N * (2 * t + 1)], ps[0:64, :]
                        )
                        nc.vector.tensor_copy(
                            qk_k[64:128, N * (2 * t + 1) : N * (2 * t + 2)],
                            ps[64:128, :],
                        )

                def qk_compute(ft, wqs):
                    """qkT block ft, monolithic (pre-head phase)."""
                    ps = psp.tile([128, N], F32, tag="s")
                    for ct in range(NCT):
                        lhs = wqs[:, 128 * ct : 128 * (ct + 1)]
                        for qh in range(2):
                            nc.tensor.matmul(
                                ps[:, 512 * qh : 512 * (qh + 1)],
                                lhs,
                                xr[:, N * ct + 512 * qh : N * ct + 512 * (qh + 1)],
                                start=(ct == 0),
                                stop=(ct == NCT - 1),
                            )
                    qk_evac(ft, ps)

                filler = collections.deque()

                def queue_qk_chunks(ft, wqs):
                    """qkT block ft as 6 filler chunks (2 matmuls each),
                    accumulating in a u-tag PSUM slot."""
                    cell = {}

                    def chunk(ct):
                        if ct == 0:
                            cell["ps"] = psp.tile(
                                [128, N], F32, tag="u", name="qk_acc"
                            )
                        ps = cell["ps"]
                        lhs = wqs[:, 128 * ct : 128 * (ct + 1)]
                        for qh in range(2):
                            nc.tensor.matmul(
                                ps[:, 512 * qh : 512 * (qh + 1)],
                                lhs,
                                xr[:, N * ct + 512 * qh : N * ct + 512 * (qh + 1)],
                                start=(ct == 0),
                                stop=(ct == NCT - 1),
                            )
                        if ct == NCT - 1:
                            qk_evac(ft, ps)

                    for ct in range(NCT):
                        filler.append(lambda ct=ct: chunk(ct))

                def v_block(m, tag="s"):
                    """v token-tile m -> vp [128, 780]: 12x(64 v cols + ones)."""
                    ps = psp.tile([128, N], F32, tag=tag, name="v_acc")
                    for ct in range(NCT):
                        lhs = xr[:, N * ct + 128 * m : N * ct + 128 * (m + 1)]
                        for nn, nw in ((0, 512), (512, 256)):
                            nc.tensor.matmul(
                                ps[:, nn : nn + nw],
                                lhs,
                                wv_t[:, C * ct + nn : C * ct + nn + nw],
                                start=(ct == 0),
                                stop=(ct == NCT - 1),
                            )
                    blk = vp_t[:, WV * m : WV * (m + 1)].rearrange(
                        "p (h c) -> p h c", c=HD + 1
                    )
                    nc.vector.tensor_copy(
                        blk[:, :, 0:HD],
                        ps[:, 0:C].rearrange("p (h c) -> p h c", c=HD),
                    )
                    nc.vector.tensor_copy(
                        blk[:, :, HD : HD + 1],
                        ones12[:].rearrange("p (h o) -> p h o", o=1),
                    )

                def head0_split():
                    """Head 0 in two waves of 4 k-tiles: scores+exp emitted
                    before that wave's v blocks, so ACT drains exps while the
                    PE computes v. Wave size matches pt_bufs."""
                    qft, po = 0, 0
                    wave = min(pt_bufs, 4)
                    ps_u = psp.tile([128, N], F32, tag="u")
                    for w0 in range(0, NTT, wave):
                        pts = []
                        for kt in range(w0, w0 + wave):
                            ps_s = psp.tile([128, N], F32, tag="s")
                            ksl = qk_k[:, 128 * kt : 128 * (kt + 1)]
                            for qh in range(2):
                                nc.tensor.matmul(
                                    ps_s[:, 512 * qh : 512 * (qh + 1)],
                                    ksl,
                                    qk_q[
                                        :,
                                        N * qft + 512 * qh : N * qft + 512 * (qh + 1),
                                    ],
                                    start=True,
                                    stop=True,
                                )
                            pt = ptp.tile([128, N], BF16, tag="pt")
                            nc.scalar.activation(pt[:], ps_s[:], EXP)
                            pts.append(pt)
                        for m in range(w0, w0 + wave):
                            v_block(m)
                        for kt in range(w0, w0 + wave):
                            vsl = vp_t[:, WV * kt : WV * kt + HD + 1]
                            for qh in range(2):
                                sl = slice(512 * qh, 512 * (qh + 1))
                                nc.tensor.matmul(
                                    ps_u[0:65, sl], vsl, pts[kt - w0][:, sl],
                                    start=(kt == 0), stop=(kt == NTT - 1),
                                )
                            if filler:
                                filler.popleft()()
                    uT = wkp.tile([128, N], F32, tag="uT", bufs=1)
                    nc.vector.tensor_copy(uT[0:65, :], ps_u[0:65, :])
                    rec_f = wkp.tile([1, N], F32, tag="recf2", bufs=1)
                    nc.vector.reciprocal(rec_f[:], uT[64:65, :])
                    bc = wkp.tile([64, N], F32, tag="bc", bufs=1)
                    nc.gpsimd.partition_broadcast(bc[:], rec_f[:])
                    nc.vector.tensor_tensor(
                        attnT[po : po + 64, N * qft : N * (qft + 1)],
                        uT[0:64, :],
                        bc[:],
                        op=MULT,
                    )

                def head(h):
                    qft, po = h // 2, 64 * (h % 2)
                    ps_u = psp.tile([128, N], F32, tag="u")

                    def attnv(kt, pt):
                        vsl = vp_t[
                            :, WV * kt + (HD + 1) * h : WV * kt + (HD + 1) * (h + 1)
                        ]
                        for qh in range(2):
                            sl = slice(512 * qh, 512 * (qh + 1))
                            nc.tensor.matmul(
                                ps_u[0:65, sl], vsl, pt[:, sl],
                                start=(kt == 0), stop=(kt == NTT - 1),
                            )

                    # attn@v lags the exps by one k-tile so its pt input is
                    # long-ready (no sem-gated PE stall / LDW prefetch break)
                    prev = None
                    for kt in range(NTT):
                        ps_s = psp.tile([128, N], F32, tag="s")
                        ksl = qk_k[
                            :, N * h + 128 * kt : N * h + 128 * (kt + 1)
                        ]
                        for qh in range(2):
                            nc.tensor.matmul(
                                ps_s[:, 512 * qh : 512 * (qh + 1)],
                                ksl,
                                qk_q[
                                    :,
                                    N * qft + 512 * qh : N * qft + 512 * (qh + 1),
                                ],
                                start=True,
                                stop=True,
                            )
                        pt = ptp.tile([128, N], BF16, tag="pt")
                        nc.scalar.activation(pt[:], ps_s[:], EXP)
                        if prev is not None:
                            attnv(kt - 1, prev)
                        prev = pt
                        if filler and (kt >= 2 or h % 2 == 1):
                            filler.popleft()()
                    attnv(NTT - 1, prev)
                    # evacuate U+den, normalize off the PE:
                    # partition_broadcast den (gpsimd) -> divide (DVE)
                    uT = wkp.tile([128, N], F32, tag="uT", bufs=1)
                    nc.vector.tensor_copy(uT[0:65, :], ps_u[0:65, :])
                    rec_f = wkp.tile([1, N], F32, tag="recf2", bufs=1)
                    nc.vector.reciprocal(rec_f[:], uT[64:65, :])
                    bc = wkp.tile([64, N], F32, tag="bc", bufs=1)
                    nc.gpsimd.partition_broadcast(bc[:], rec_f[:])
                    nc.vector.tensor_tensor(
                        attnT[po : po + 64, N * qft : N * (qft + 1)],
                        uT[0:64, :],
                        bc[:],
                        op=MULT,
                    )

                # pre-head phase: blocks 0,6; head 0 split (v inside)
                qk_compute(0, w_first)
                qk_compute(6, w_second)

                # heads with deadline-scheduled qk fillers:
                # pair t (blocks t, 6+t) loads at head 2t-3, chunks during
                # heads 2t-2 / 2t-1, needed by head 2t.
                loads = {}
                loads[0] = (wq_load(1), wq_load(7))  # before head 0
                for h in range(H):
                    t = h // 2 + 1
                    if h % 2 == 0 and t <= 5:
                        wa, wb = loads.pop(h)
                        queue_qk_chunks(t, wa)
                        queue_qk_chunks(6 + t, wb)
                        if t + 1 <= 5:
                            loads[h + 2] = (wq_load(t + 1), wq_load(7 + t))
                    if h == 6:
                        nc.sync.dma_start(
                            wps[1 - par][:].rearrange("p (ct f) -> p ct f", f=C),
                            wp_d[:].rearrange("(ct p) f -> p ct f", p=128),
                        )
                    if h == 0:
                        head0_split()
                    else:
                        head(h)
                while filler:
                    filler.popleft()()

                # projection
                for m in range(NTT):
                    ps_o = psp.tile([128, N], F32, tag="s")
                    for ct in range(NCT - 1):
                        lhs = attnT[:, N * ct + 128 * m : N * ct + 128 * (m + 1)]
                        for nn, nw in ((0, 512), (512, 256)):
                            nc.tensor.matmul(
                                ps_o[:, nn : nn + nw],
                                lhs,
                                wp_t[:, C * ct + nn : C * ct + nn + nw],
                                start=(ct == 0),
                                stop=False,
                            )
                    ct = NCT - 1
                    lhs = attnT[:, N * ct + 128 * m : N * ct + 128 * (m + 1)]
                    for nn, nw in ((0, 512), (512, 256)):
                        nc.tensor.matmul(
                            ps_o[:, nn : nn + nw],
                            lhs,
                            wp_t[:, C * ct + nn : C * ct + nn + nw],
                            start=False,
                            stop=True,
                        )
                    o_sb = wkp.tile([128, C], F32, tag="osb", bufs=2)
                    nc.vector.tensor_tensor(
                        o_sb[:], ps_o[:, 0:C], bias_sb[:], op=mybir.AluOpType.add
                    )
                    nc.sync.dma_start(out_d[128 * m : 128 * (m + 1), :], o_sb[:])

            warmup()
            if reps:
                assert reps % 2 == 0, reps
                with tc.For_i(0, reps // 2, 1):
                    emit(0)
                    emit(1)
            else:
                emit(0)

    nc.compile()
    return nc


_CACHE = {}


def _get_nc():
    if "nc" not in _CACHE:
        _CACHE["nc"] = _build()
    return _CACHE["nc"]


def _host_prep(w_qkv, w_proj, b_proj):
    import ml_dtypes

    bf = ml_dtypes.bfloat16
    ws = np.asarray(w_qkv, dtype=np.float32).copy()
    ws[0:C] *= SCALE
    wt = np.ascontiguousarray(ws.T)  # [768, 2304]
    wqb = np.ascontiguousarray(
        wt[:, : 2 * C].reshape(C, NQK, 128).transpose(1, 0, 2)
    ).astype(bf)
    wv = np.ascontiguousarray(wt[:, 2 * C :]).astype(bf)
    wp = np.ascontiguousarray(np.asarray(w_proj, dtype=np.float32).T).astype(bf)
    bp = np.ascontiguousarray(
        np.tile(np.asarray(b_proj, dtype=np.float32)[None, :], (128, 1))
    )
    return wqb, wv, wp, bp


def _prep_xT(xb):
    import ml_dtypes

    return np.ascontiguousarray(xb.T).astype(ml_dtypes.bfloat16)


def kernel(x, w_qkv, w_proj, b_proj):
    x = np.asarray(x, dtype=np.float32)
    assert x.shape == (B, N, C), x.shape
    wqb, wv, wp, bp = _host_prep(w_qkv, w_proj, b_proj)
    in_maps = [
        {
            "xT": _prep_xT(x[b]),
            "wqb": wqb,
            "wv": wv,
            "wp": wp,
            "bp": bp,
        }
        for b in range(B)
    ]
    nc = _get_nc()
    res = bass_utils.run_bass_kernel_spmd(nc, in_maps, core_ids=list(range(B)))
    return np.stack([np.asarray(res.results[b]["out"]) for b in range(B)]).astype(
        np.float32
    )

